# revision 1
# baseline (speedup 1.0000x reference)
"""Trainium2 Bass kernel for nn_CausalFreqMixer (causal depthwise long-conv mixer).

Math: p = x @ W + b -> [v, g1, g2] ; for each stage: v = irfft(rfft(v,4096)*Hs)[:L] * gs.

Implementation: full-DFT-as-matmul. The per-channel frequency filter is a
pointwise multiply; the DFT/IDFT along seq are channel-shared dense matrices,
so they run on the TensorEngine as [2048x2048] @ [2048x512] matmuls with the
natural [seq-partition, channel-free] layout (no transposes anywhere).

Packed-real spectrum: bins 0..2047 with Re(X[2048]) (Nyquist) packed into the
Im slot of bin 0. Forward and inverse both use the SAME two symmetric
matrices, Fc[t,k]=cos(2*pi*t*k/4096) and S0[t,k]=-sin(2*pi*t*k/4096); all
irfft scaling is folded into the host-precomputed filter spectra, and the
packed DC/Nyquist slots are handled with cheap rank-1 matmul fixups.

Sharding: 8 cores = 4 batch samples x 2 channel halves. Zero cross-core
communication; the projection weight is column-sharded to each core's
channels.
"""
import numpy as np
from contextlib import ExitStack

import concourse.bass as bass
import concourse.bacc as bacc
import concourse.tile as tile
import concourse.mybir as mybir
from concourse.bass_utils import run_bass_kernel_spmd

try:
    import ml_dtypes
    _NP_BF16 = ml_dtypes.bfloat16
except ImportError:  # pragma: no cover
    _NP_BF16 = None

# ---- problem constants (hardcoded per contract) ----
B, L, D = 4, 2048, 1024
NFFT = 2 * L
ORDER = 2
N_CORES = 8
C = D // (N_CORES // B)      # 512 channels per core
NT = L // 128                # 16 seq tiles
ND = D // 128                # 8 contraction tiles for the projection
E = (ORDER + 1) * C          # 1536 projected columns per core

# ---- tuning knobs (defaults used by kernel()) ----
MM_MODE = "f32r"             # "f32" | "f32r" | "bf16"
REPEAT = 1

_ALU = mybir.AluOpType


def _mdt(mode):
    if mode == "bf16":
        return mybir.dt.bfloat16
    if mode == "f16":
        return mybir.dt.float16
    if mode == "f32r":
        return mybir.dt.float32r
    return mybir.dt.float32


def _np_mdt(mode):
    if mode == "bf16":
        return _NP_BF16
    if mode == "f16":
        return np.float16
    return np.float32


def _emit(nc, mode, repeat):
    mdt = _mdt(mode)
    f32 = mybir.dt.float32

    def mm(out, lhsT, rhs, start, stop):
        nc.tensor.matmul(out, lhsT, rhs, start=start, stop=stop)

    xt = nc.dram_tensor("xt", [D, L], mdt, kind="ExternalInput").ap()
    w = nc.dram_tensor("w", [D, E], mdt, kind="ExternalInput").ap()
    bias = nc.dram_tensor("bias", [1, E], f32, kind="ExternalInput").ap()
    fct = nc.dram_tensor("fct", [NT, NT, 128, 128], mdt, kind="ExternalInput").ap()
    s0t = nc.dram_tensor("s0t", [NT, NT, 128, 128], mdt, kind="ExternalInput").ap()
    altc = nc.dram_tensor("altc", [L], mdt, kind="ExternalInput").ap()
    altr = nc.dram_tensor("altr", [1, 128], mdt, kind="ExternalInput").ap()
    ha = nc.dram_tensor("ha", [ORDER, L, C], f32, kind="ExternalInput").ap()
    hb = nc.dram_tensor("hb", [ORDER, L, C], f32, kind="ExternalInput").ap()
    hd0 = nc.dram_tensor("hd0", [1, ORDER * C], f32, kind="ExternalInput").ap()
    out_d = nc.dram_tensor("out", [L, C], f32, kind="ExternalOutput").ap()

    xt_r = xt.rearrange("(kd p) l -> p kd l", p=128)
    w_r = w.rearrange("(kd p) e -> p kd e", p=128)
    altc_r = altc.rearrange("(j p) -> p j", p=128)

    with tile.TileContext(nc) as tc:
        with ExitStack() as ctx:
            consts = ctx.enter_context(tc.tile_pool(name="consts", bufs=1))
            bigs = ctx.enter_context(tc.tile_pool(name="bigs", bufs=1))
            dram = ctx.enter_context(tc.tile_pool(name="dram", bufs=1, space="DRAM"))
            psum = ctx.enter_context(tc.tile_pool(name="psum", bufs=6, space="PSUM"))
            tpool = ctx.enter_context(tc.tile_pool(name="tpool", bufs=6))

            altc_s = consts.tile([128, NT], mdt)
            nc.sync.dma_start(out=altc_s, in_=altc_r)
            altr_s = consts.tile([1, 128], mdt)
            nc.sync.dma_start(out=altr_s, in_=altr)
            hd0_s = consts.tile([1, ORDER * C], f32)
            nc.sync.dma_start(out=hd0_s, in_=hd0)
            bias_s = consts.tile([128, E], f32)
            nc.sync.dma_start(out=bias_s, in_=bias.partition_broadcast(128))

            v_buf = bigs.tile([128, NT, C], mdt)
            yr_buf = bigs.tile([128, NT, C], mdt)
            yi_buf = bigs.tile([128, NT, C], mdt)

            p_scr = dram.tile([ORDER, L, C], f32)

            for _rep in range(repeat):
                # ---------------- projection ----------------
                with tc.tile_pool(name="wproj", bufs=1) as wproj, \
                        tc.tile_pool(name="xtp", bufs=2) as xtp:
                    w_s = wproj.tile([128, ND, E], mdt)
                    for kd in range(ND):
                        nc.sync.dma_start(out=w_s[:, kd, :], in_=w_r[:, kd, :])
                    for lt in range(NT):
                        xts = xtp.tile([128, ND, 128], mdt, tag="xts")
                        nc.sync.dma_start(
                            out=xts, in_=xt_r[:, :, lt * 128:(lt + 1) * 128]
                        )
                        for ch in range(ORDER + 1):
                            ps = psum.tile([128, C], f32, tag="ps")
                            for kd in range(ND):
                                mm(ps, xts[:, kd, :], w_s[:, kd, ch * C:(ch + 1) * C],
                                   start=(kd == 0), stop=(kd == ND - 1))
                            bsl = bias_s[:, ch * C:(ch + 1) * C]
                            if ch == 0:
                                nc.vector.scalar_tensor_tensor(
                                    out=v_buf[:, lt, :], in0=ps, scalar=1.0,
                                    in1=bsl, op0=_ALU.mult, op1=_ALU.add)
                            else:
                                g = tpool.tile([128, C], f32, tag="t")
                                nc.vector.scalar_tensor_tensor(
                                    out=g, in0=ps, scalar=1.0,
                                    in1=bsl, op0=_ALU.mult, op1=_ALU.add)
                                nc.sync.dma_start(
                                    out=p_scr[ch - 1, lt * 128:(lt + 1) * 128, :],
                                    in_=g)

                # ---------------- stages ----------------
                wstage_cm = tc.tile_pool(name="wstage", bufs=3)
                wstage = wstage_cm.__enter__()
                xpool_cm = tc.tile_pool(name="xpool", bufs=3)
                xpool = xpool_cm.__enter__()
                hpool_cm = tc.tile_pool(name="hpool", bufs=3)
                hpool = hpool_cm.__enter__()
                for st in range(ORDER):
                    # forward DFT + pointwise, per bin-tile m
                    for m in range(NT):
                        wf = wstage.tile([128, NT, 128], mdt, tag="wf")
                        nc.sync.dma_start(out=wf, in_=fct[m].transpose([1, 0, 2]))
                        ws = wstage.tile([128, NT, 128], mdt, tag="ws")
                        nc.sync.dma_start(out=ws, in_=s0t[m].transpose([1, 0, 2]))

                        psxr = psum.tile([128, C], f32, tag="ps")
                        for j in range(NT):
                            mm(psxr, wf[:, j, :], v_buf[:, j, :],
                               start=(j == 0), stop=(j == NT - 1))
                        psxi = psum.tile([128, C], f32, tag="ps")
                        for j in range(NT - 1):
                            mm(psxi, ws[:, j, :], v_buf[:, j, :],
                               start=(j == 0), stop=False)
                        if m == 0:
                            # packed-Nyquist row: Xi[0] += sum_t (-1)^t v[t]
                            for j in range(NT):
                                mm(psxi[0:1, :], altc_s[:, j:j + 1], v_buf[:, j, :],
                                   start=False, stop=False)
                        mm(psxi, ws[:, NT - 1, :], v_buf[:, NT - 1, :],
                           start=False, stop=True)

                        xr = xpool.tile([128, C], f32, tag="x")
                        nc.scalar.copy(out=xr, in_=psxr)
                        xi = xpool.tile([128, C], f32, tag="x")
                        nc.scalar.copy(out=xi, in_=psxi)

                        hat = hpool.tile([128, C], f32, tag="ha")
                        nc.sync.dma_start(out=hat, in_=ha[st, m * 128:(m + 1) * 128, :])
                        hbt = hpool.tile([128, C], f32, tag="hb")
                        nc.sync.dma_start(out=hbt, in_=hb[st, m * 128:(m + 1) * 128, :])

                        t1 = tpool.tile([128, C], f32, tag="t")
                        nc.vector.tensor_mul(t1, xr, hat)
                        t2 = tpool.tile([128, C], f32, tag="t")
                        nc.vector.tensor_mul(t2, xi, hbt)
                        nc.vector.tensor_sub(yr_buf[:, m, :], t1, t2)
                        t3 = tpool.tile([128, C], f32, tag="t")
                        nc.vector.tensor_mul(t3, xr, hbt)
                        t4 = tpool.tile([128, C], f32, tag="t")
                        nc.vector.tensor_mul(t4, xi, hat)
                        nc.vector.tensor_add(yi_buf[:, m, :], t3, t4)
                        if m == 0:
                            # packed slot: Yi[0] = Xi[0] * ReH[Nyq] * 1/N
                            nc.vector.tensor_mul(
                                yi_buf[0:1, 0, :], xi[0:1, :],
                                hd0_s[0:1, st * C:(st + 1) * C])

                    # inverse DFT + gate, per time-tile mt
                    for mt in range(NT):
                        wfi = wstage.tile([128, NT, 128], mdt, tag="wf")
                        nc.sync.dma_start(out=wfi, in_=fct[mt].transpose([1, 0, 2]))
                        wsi = wstage.tile([128, NT, 128], mdt, tag="ws")
                        nc.sync.dma_start(out=wsi, in_=s0t[mt].transpose([1, 0, 2]))

                        psc = psum.tile([128, C], f32, tag="ps")
                        for j in range(NT):
                            mm(psc, wfi[:, j, :], yr_buf[:, j, :],
                               start=(j == 0), stop=False)
                        for j in range(NT):
                            mm(psc, wsi[:, j, :], yi_buf[:, j, :],
                               start=False, stop=False)
                        # packed slot contribution: conv[t] += (-1)^t * Yi[0]
                        mm(psc, altr_s[0:1, :],
                           yi_buf[0:1, 0, :], start=False, stop=True)

                        gin = tpool.tile([128, C], f32, tag="t")
                        nc.sync.dma_start(
                            out=gin, in_=p_scr[st, mt * 128:(mt + 1) * 128, :])
                        if st < ORDER - 1:
                            nc.vector.scalar_tensor_tensor(
                                out=v_buf[:, mt, :], in0=psc, scalar=1.0,
                                in1=gin, op0=_ALU.mult, op1=_ALU.mult)
                        else:
                            og = tpool.tile([128, C], f32, tag="t")
                            nc.vector.scalar_tensor_tensor(
                                out=og, in0=psc, scalar=1.0,
                                in1=gin, op0=_ALU.mult, op1=_ALU.mult)
                            nc.sync.dma_start(
                                out=out_d[mt * 128:(mt + 1) * 128, :], in_=og)
                hpool_cm.__exit__(None, None, None)
                xpool_cm.__exit__(None, None, None)
                wstage_cm.__exit__(None, None, None)


_PROGRAMS = {}


def build_program(mode=None, repeat=None):
    mode = MM_MODE if mode is None else mode
    repeat = REPEAT if repeat is None else repeat
    key = (mode, repeat)
    if key not in _PROGRAMS:
        nc = bacc.Bacc("TRN2", target_bir_lowering=False, debug=False,
                       enable_asserts=False, num_devices=N_CORES)
        _emit(nc, mode, repeat)
        nc.compile()
        _PROGRAMS[key] = nc
    return _PROGRAMS[key]


_TABLES = {}


def host_tables(mode=None):
    """Shared DFT matrices, pre-tiled as [m, j, p, k] = M[128j+p, 128m+k]."""
    mode = MM_MODE if mode is None else mode
    if mode not in _TABLES:
        npdt = _np_mdt(mode)
        t = np.arange(L, dtype=np.float64)
        ang = (2.0 * np.pi / NFFT) * np.outer(t, t)
        fc = np.cos(ang)
        s0 = -np.sin(ang)

        def tile4(mat):
            return np.ascontiguousarray(
                mat.reshape(NT, 128, NT, 128).transpose(2, 0, 1, 3).astype(npdt))

        alt = ((-1.0) ** np.arange(L))
        _TABLES[mode] = {
            "fct": tile4(fc),
            "s0t": tile4(s0),
            "altc": alt.astype(npdt),
            "altr": np.ascontiguousarray(alt[:128].reshape(1, 128).astype(npdt)),
        }
    return _TABLES[mode]


def filter_spectra(filter_time):
    """Packed, scale-folded filter spectra per stage: (A, B, d0) with
    Yr = Xr*A - Xi*B ; Yi = Xr*B + Xi*A except Yi[0] = Xi[0]*d0."""
    out = []
    for stg in range(ORDER):
        h = np.asarray(filter_time[stg, 0], dtype=np.float64)   # [L, D]
        H = np.fft.rfft(h, n=NFFT, axis=0)                       # [L+1, D]
        s = np.full((L, 1), 2.0 / NFFT)
        s[0, 0] = 1.0 / NFFT
        A = (H[:L].real * s).astype(np.float32)
        Bm = (H[:L].imag * s).astype(np.float32)
        Bm[0, :] = 0.0
        d0 = (H[L].real / NFFT).astype(np.float32)               # [D]
        out.append((A, Bm, d0))
    return out


def make_in_maps(x, proj_w, proj_b, filter_time, mode=None):
    mode = MM_MODE if mode is None else mode
    npdt = _np_mdt(mode)
    tables = host_tables(mode)
    specs = filter_spectra(filter_time)
    in_maps = []
    for core in range(N_CORES):
        b, half = divmod(core, N_CORES // B)
        c0 = half * C
        cols = np.concatenate(
            [np.arange(s * D + c0, s * D + c0 + C) for s in range(ORDER + 1)])
        ha = np.stack([specs[stg][0][:, c0:c0 + C] for stg in range(ORDER)])
        hb = np.stack([specs[stg][1][:, c0:c0 + C] for stg in range(ORDER)])
        hd0 = np.concatenate(
            [specs[stg][2][c0:c0 + C] for stg in range(ORDER)]).reshape(1, ORDER * C)
        in_maps.append({
            "xt": np.ascontiguousarray(np.asarray(x[b]).T.astype(npdt)),
            "w": np.ascontiguousarray(np.asarray(proj_w)[:, cols].astype(npdt)),
            "bias": np.ascontiguousarray(
                np.asarray(proj_b)[cols].astype(np.float32).reshape(1, E)),
            "fct": tables["fct"],
            "s0t": tables["s0t"],
            "altc": tables["altc"],
            "altr": tables["altr"],
            "ha": np.ascontiguousarray(ha.astype(np.float32)),
            "hb": np.ascontiguousarray(hb.astype(np.float32)),
            "hd0": np.ascontiguousarray(hd0.astype(np.float32)),
        })
    return in_maps


def gather_out(results):
    out = np.zeros((B, L, D), dtype=np.float32)
    for core in range(N_CORES):
        b, half = divmod(core, N_CORES // B)
        c0 = half * C
        out[b, :, c0:c0 + C] = results[core]["out"]
    return out


def kernel(x, proj_w, proj_b, filter_time):
    # Pull inputs to host numpy up front: device->host transfers must happen
    # BEFORE the bass NEFF executes (exec can leave the PJRT device in a
    # state where later transfers of pre-existing device arrays fail).
    x = np.asarray(x)
    proj_w = np.asarray(proj_w)
    proj_b = np.asarray(proj_b)
    filter_time = np.asarray(filter_time)
    nc = build_program()
    in_maps = make_in_maps(x, proj_w, proj_b, filter_time)
    res = run_bass_kernel_spmd(nc, in_maps, list(range(N_CORES)))
    return gather_out(res.results)



# revision 2
# speedup vs baseline: 880.1966x; 880.1966x over previous
"""Trainium2 Bass kernel for nn_CausalFreqMixer — CRT-split edition.

Math: p = x @ W + b -> [v, g1, g2]; per stage: v = causalconv(v, h_s) * g_s.

The length-2048 causal conv (first half of a 4096-point circular conv with
the zero-padded signal) is CRT-split into five sub-convolutions driven by
channel-shared transform matmuls + per-channel pointwise spectra:

    A: cyclic-512  of a1 = v0+v1+v2+v3          (256 packed bins, own mats)
    B: nega-512    of a2 = v0-v1+v2-v3          (256 complex bins, own mats)
    F: nega-1024   of [v0-v2, v1-v3] via mod(z^512-i)   (512 complex bins)
    D: mod(z^512-s) of (v0+i v2)+s(v1+i v3), s=e^{i pi/4}
    E: mod(z^512+s) of (v0+i v2)-s(v1+i v3)

F, D, E all share ONE set of alpha=1/8 twisted-DFT matrices through cheap
per-partition time-domain twists (e^{i pi t/2048} / e^{i pi t/512}). This
cuts stage matmul MACs 2.3x vs the full [2048x2048] DFT, with no
transposes, and the whole matrix set is SBUF-resident in bf16
(40 KB/partition). PSUM accumulation stays f32; elementwise work is split
across DVE / Pool / Activation.

Sharding: 8 cores = 4 batch x 2 channel halves. Zero cross-core traffic.
"""
import numpy as np
from contextlib import ExitStack

import concourse.bass as bass
import concourse.bacc as bacc
import concourse.tile as tile
import concourse.mybir as mybir
from concourse.bass_utils import run_bass_kernel_spmd

try:
    import ml_dtypes
    _NP_BF16 = ml_dtypes.bfloat16
except ImportError:  # pragma: no cover
    _NP_BF16 = None

# ---- problem constants ----
B, L, D = 4, 2048, 1024
ORDER = 2
N_CORES = 8
C = D // (N_CORES // B)      # 512 channels per core
NT = L // 128                # 16 seq tiles
ND = D // 128                # 8 contraction tiles for the projection
E = (ORDER + 1) * C          # 1536 projected columns per core
R2 = float(1.0 / np.sqrt(2.0))

MM_MODE = "f16"
REPEAT = 1

_ALU = mybir.AluOpType

# ---- transform-matrix inventory: name -> (out_rows, in_cols) ----
_MATS_SHAPES = {
    "AfC": (256, 512), "AfS": (256, 512),
    "BfC": (256, 512), "BfS": (256, 512),
    "DfR": (512, 512), "DfI": (512, 512), "DfIN": (512, 512),
    "AiC": (512, 256), "AiS": (512, 256),
    "BiC": (512, 256), "BiS": (512, 256),
    "DiR": (512, 512), "DiI": (512, 512), "DiIN": (512, 512),
}
OFF = {}
_n = 0
for _name, (_o, _i) in _MATS_SHAPES.items():
    OFF[_name] = _n
    _n += (_o // 128) * (_i // 128)
N_MTILES = _n  # 160 tiles -> 40 KB/partition in bf16

# spec_buf / hspec slot map (32 slots of [128, C])
SA_R, SA_I = 0, 2
SB_R, SB_I = 4, 6
SF_R, SF_I = 8, 12
SD_R, SD_I = 16, 20
SE_R, SE_I = 24, 28
# combos slot map (32 slots)
K_A1, K_A2, K_FR, K_FI, K_DR, K_DI, K_ER, K_EI = 0, 4, 8, 12, 16, 20, 24, 28


def _mdt(mode):
    if mode == "bf16":
        return mybir.dt.bfloat16
    if mode == "f16":
        return mybir.dt.float16
    if mode == "f32r":
        return mybir.dt.float32r
    return mybir.dt.float32


def _np_mdt(mode):
    if mode == "bf16":
        return _NP_BF16
    if mode == "f16":
        return np.float16
    return np.float32


class _EngMux:
    """Round-robin 2-input elementwise ops over DVE (2 slots) + Pool (1)."""

    def __init__(self, nc):
        self.engs = [nc.vector, nc.vector, nc.vector, nc.gpsimd]
        self.i = 0

    def __call__(self):
        e = self.engs[self.i % len(self.engs)]
        self.i += 1
        return e


def _emit(nc, mode, repeat):
    mdt = _mdt(mode)
    f32 = mybir.dt.float32

    def mm(out, lhsT, rhs, start, stop):
        nc.tensor.matmul(out, lhsT, rhs, start=start, stop=stop)

    xt = nc.dram_tensor("xt", [D, L], mdt, kind="ExternalInput").ap()
    w = nc.dram_tensor("w", [D, E], mdt, kind="ExternalInput").ap()
    bias = nc.dram_tensor("bias", [1, E], mdt, kind="ExternalInput").ap()
    mats = nc.dram_tensor("mats", [128, N_MTILES * 128], mdt,
                          kind="ExternalInput").ap()
    etw = nc.dram_tensor("etw", [128, 16], f32, kind="ExternalInput").ap()
    hspec = nc.dram_tensor("hspec", [ORDER * 32, 128, C], mdt,
                           kind="ExternalInput").ap()
    hfix = nc.dram_tensor("hfix", [1, ORDER * C], f32,
                          kind="ExternalInput").ap()
    out_d = nc.dram_tensor("out", [L, C], f32, kind="ExternalOutput").ap()

    xt_r = xt.rearrange("(kd p) l -> p kd l", p=128)
    w_r = w.rearrange("(kd p) e -> p kd e", p=128)

    with tile.TileContext(nc) as tc:
        with ExitStack() as ctx:
            consts = ctx.enter_context(tc.tile_pool(name="consts", bufs=1))
            bigs = ctx.enter_context(tc.tile_pool(name="bigs", bufs=1))
            dram = ctx.enter_context(tc.tile_pool(name="dram", bufs=1, space="DRAM"))
            psum = ctx.enter_context(tc.tile_pool(name="psum", bufs=8, space="PSUM"))
            tpool = ctx.enter_context(tc.tile_pool(name="tpool", bufs=8))
            ipool = ctx.enter_context(tc.tile_pool(name="ipool", bufs=18))
            hpool = ctx.enter_context(tc.tile_pool(name="hpool", bufs=6))
            gpool = ctx.enter_context(tc.tile_pool(name="gpool", bufs=3))

            mats_s = consts.tile([128, N_MTILES * 128], mdt)
            nc.sync.dma_start(out=mats_s, in_=mats)
            etw_s = consts.tile([128, 16], f32)
            nc.sync.dma_start(out=etw_s, in_=etw)
            hfix_s = consts.tile([1, ORDER * C], f32)
            nc.sync.dma_start(out=hfix_s, in_=hfix)

            def MT(name, ot, it):
                n_it = _MATS_SHAPES[name][1] // 128
                idx = OFF[name] + ot * n_it + it
                return mats_s[:, idx * 128:(idx + 1) * 128]

            v_buf = bigs.tile([128, NT, C], mdt)
            spec = bigs.tile([128, 32, C], mdt)
            combos = bigs.tile([128, 32, C], mdt)

            p_scr = dram.tile([ORDER, L, C], mdt)

            for _rep in range(repeat):
                # ---------------- projection ----------------
                with tc.tile_pool(name="wproj", bufs=1) as wproj, \
                        tc.tile_pool(name="xtp", bufs=2) as xtp, \
                        tc.tile_pool(name="bpool", bufs=1) as bpool:
                    bias_s = bpool.tile([128, E], mdt)
                    nc.sync.dma_start(out=bias_s,
                                      in_=bias.partition_broadcast(128))
                    w_s = wproj.tile([128, ND, E], mdt)
                    for kd in range(ND):
                        nc.sync.dma_start(out=w_s[:, kd, :], in_=w_r[:, kd, :])
                    for lt in range(NT):
                        xts = xtp.tile([128, ND, 128], mdt, tag="xts")
                        nc.sync.dma_start(
                            out=xts, in_=xt_r[:, :, lt * 128:(lt + 1) * 128])
                        for ch in range(ORDER + 1):
                            ps = psum.tile([128, C], f32, tag="ps")
                            for kd in range(ND):
                                mm(ps, xts[:, kd, :],
                                   w_s[:, kd, ch * C:(ch + 1) * C],
                                   start=(kd == 0), stop=(kd == ND - 1))
                            bsl = bias_s[:, ch * C:(ch + 1) * C]
                            if ch == 0:
                                nc.vector.scalar_tensor_tensor(
                                    out=v_buf[:, lt, :], in0=ps, scalar=1.0,
                                    in1=bsl, op0=_ALU.mult, op1=_ALU.add)
                            else:
                                g = tpool.tile([128, C], mdt, tag="tb")
                                nc.vector.scalar_tensor_tensor(
                                    out=g, in0=ps, scalar=1.0,
                                    in1=bsl, op0=_ALU.mult, op1=_ALU.add)
                                nc.sync.dma_start(
                                    out=p_scr[ch - 1,
                                              lt * 128:(lt + 1) * 128, :],
                                    in_=g)

                # ---------------- stages ----------------
                for st in range(ORDER):
                    em = _EngMux(nc)

                    # combos pass 1: a1/a2 so A/B forwards start immediately
                    for j in range(4):
                        v0 = v_buf[:, j, :]
                        v1 = v_buf[:, 4 + j, :]
                        v2 = v_buf[:, 8 + j, :]
                        v3 = v_buf[:, 12 + j, :]
                        P02 = tpool.tile([128, C], mdt, tag="tb")
                        P13 = tpool.tile([128, C], mdt, tag="tb")
                        nc.vector.tensor_add(P02, v0, v2)
                        nc.gpsimd.tensor_add(P13, v1, v3)
                        nc.vector.tensor_add(combos[:, K_A1 + j, :], P02, P13)
                        nc.vector.tensor_sub(combos[:, K_A2 + j, :], P02, P13)

                    def pointwise(psr, psi, slot_r, slot_i, fix=False):
                        hr = hpool.tile([128, C], mdt, tag="h")
                        nc.sync.dma_start(out=hr, in_=hspec[st * 32 + slot_r])
                        hi = hpool.tile([128, C], mdt, tag="h")
                        nc.sync.dma_start(out=hi, in_=hspec[st * 32 + slot_i])
                        t1 = tpool.tile([128, C], mdt, tag="tb")
                        t2 = tpool.tile([128, C], mdt, tag="tb")
                        nc.vector.tensor_mul(t1, psr, hr)
                        nc.vector.tensor_mul(t2, psi, hi)
                        t3 = tpool.tile([128, C], mdt, tag="tb")
                        t4 = tpool.tile([128, C], mdt, tag="tb")
                        nc.vector.tensor_mul(t3, psr, hi)
                        nc.vector.tensor_mul(t4, psi, hr)
                        nc.gpsimd.tensor_sub(spec[:, slot_r, :], t1, t2)
                        nc.gpsimd.tensor_add(spec[:, slot_i, :], t3, t4)
                        if fix:
                            # packed Nyquist: Yi[0] = Xi[0] * H_nyq
                            nc.vector.tensor_mul(
                                spec[0:1, slot_i, :], psi[0:1, :],
                                hfix_s[0:1, st * C:(st + 1) * C])

                    def inv2(nmC, nmS, sr, si, n_it, ot):
                        ps = psum.tile([128, C], f32, tag="ps")
                        for it in range(n_it):
                            mm(ps, MT(nmC, ot, it), spec[:, sr + it, :],
                               start=(it == 0), stop=False)
                        for it in range(n_it):
                            mm(ps, MT(nmS, ot, it), spec[:, si + it, :],
                               start=False, stop=(it == n_it - 1))
                        return ps

                    # ---- A and B forwards ----
                    for nmC, nmS, k_in, s_r, s_i, fx in (
                            ("AfC", "AfS", K_A1, SA_R, SA_I, True),
                            ("BfC", "BfS", K_A2, SB_R, SB_I, False)):
                        for ot in range(2):
                            psr = psum.tile([128, C], f32, tag="ps")
                            for it in range(4):
                                mm(psr, MT(nmC, ot, it),
                                   combos[:, k_in + it, :],
                                   start=(it == 0), stop=(it == 3))
                            psi = psum.tile([128, C], f32, tag="ps")
                            for it in range(4):
                                mm(psi, MT(nmS, ot, it),
                                   combos[:, k_in + it, :],
                                   start=(it == 0), stop=(it == 3))
                            pointwise(psr, psi, s_r + ot, s_i + ot,
                                      fix=(fx and ot == 0))

                    # ---- A and B inverses (stored for combine) ----
                    iAs, iBs = [], []
                    for j in range(4):
                        pA = inv2("AiC", "AiS", SA_R, SA_I, 2, j)
                        iA = ipool.tile([128, C], mdt, tag="iv")
                        nc.scalar.copy(out=iA, in_=pA)
                        iAs.append(iA)
                        pB = inv2("BiC", "BiS", SB_R, SB_I, 2, j)
                        iB = ipool.tile([128, C], mdt, tag="iv")
                        nc.scalar.copy(out=iB, in_=pB)
                        iBs.append(iB)

                    # combos pass 2: F/D/E inputs
                    for j in range(4):
                        v0 = v_buf[:, j, :]
                        v1 = v_buf[:, 4 + j, :]
                        v2 = v_buf[:, 8 + j, :]
                        v3 = v_buf[:, 12 + j, :]
                        cosE = etw_s[:, j:j + 1]
                        sinE = etw_s[:, 4 + j:5 + j]
                        cosF = etw_s[:, 8 + j:9 + j]
                        sinF = etw_s[:, 12 + j:13 + j]
                        b_lo = tpool.tile([128, C], mdt, tag="tb")
                        b_hi = tpool.tile([128, C], mdt, tag="tb")
                        P13b = tpool.tile([128, C], mdt, tag="tb")
                        nc.vector.tensor_sub(b_lo, v0, v2)
                        nc.vector.tensor_sub(b_hi, v1, v3)
                        nc.gpsimd.tensor_add(P13b, v1, v3)
                        nc.vector.scalar_tensor_tensor(
                            out=combos[:, K_DR + j, :], in0=b_hi, scalar=R2,
                            in1=v0, op0=_ALU.mult, op1=_ALU.add)
                        nc.vector.scalar_tensor_tensor(
                            out=combos[:, K_DI + j, :], in0=P13b, scalar=R2,
                            in1=v2, op0=_ALU.mult, op1=_ALU.add)
                        ere = tpool.tile([128, C], mdt, tag="tb")
                        eim = tpool.tile([128, C], mdt, tag="tb")
                        nc.vector.scalar_tensor_tensor(
                            out=ere, in0=b_hi, scalar=-R2,
                            in1=v0, op0=_ALU.mult, op1=_ALU.add)
                        nc.vector.scalar_tensor_tensor(
                            out=eim, in0=P13b, scalar=-R2,
                            in1=v2, op0=_ALU.mult, op1=_ALU.add)
                        ts1 = tpool.tile([128, C], mdt, tag="tb")
                        ts2 = tpool.tile([128, C], mdt, tag="tb")
                        nc.scalar.mul(ts1, eim, sinE)
                        nc.scalar.mul(ts2, ere, sinE)
                        nc.vector.scalar_tensor_tensor(
                            out=combos[:, K_ER + j, :], in0=ere, scalar=cosE,
                            in1=ts1, op0=_ALU.mult, op1=_ALU.subtract)
                        nc.vector.scalar_tensor_tensor(
                            out=combos[:, K_EI + j, :], in0=eim, scalar=cosE,
                            in1=ts2, op0=_ALU.mult, op1=_ALU.add)
                        ts3 = tpool.tile([128, C], mdt, tag="tb")
                        ts4 = tpool.tile([128, C], mdt, tag="tb")
                        nc.scalar.mul(ts3, b_hi, sinF)
                        nc.scalar.mul(ts4, b_lo, sinF)
                        nc.vector.scalar_tensor_tensor(
                            out=combos[:, K_FR + j, :], in0=b_lo, scalar=cosF,
                            in1=ts3, op0=_ALU.mult, op1=_ALU.subtract)
                        nc.vector.scalar_tensor_tensor(
                            out=combos[:, K_FI + j, :], in0=b_hi, scalar=cosF,
                            in1=ts4, op0=_ALU.mult, op1=_ALU.add)

                    # ---- F / D / E forwards (shared alpha=1/8 mats) ----
                    def fwd_c(k_re, s_re):
                        k_im, s_im = k_re + 4, s_re + 4
                        for ot in range(4):
                            psr = psum.tile([128, C], f32, tag="ps")
                            for it in range(4):
                                mm(psr, MT("DfR", ot, it),
                                   combos[:, k_re + it, :],
                                   start=(it == 0), stop=False)
                            for it in range(4):
                                mm(psr, MT("DfIN", ot, it),
                                   combos[:, k_im + it, :],
                                   start=False, stop=(it == 3))
                            psi = psum.tile([128, C], f32, tag="ps")
                            for it in range(4):
                                mm(psi, MT("DfI", ot, it),
                                   combos[:, k_re + it, :],
                                   start=(it == 0), stop=False)
                            for it in range(4):
                                mm(psi, MT("DfR", ot, it),
                                   combos[:, k_im + it, :],
                                   start=False, stop=(it == 3))
                            pointwise(psr, psi, s_re + ot, s_im + ot)

                    fwd_c(K_FR, SF_R)
                    fwd_c(K_DR, SD_R)
                    fwd_c(K_ER, SE_R)

                    # ---- inverse + combine per 128-row block j ----
                    for j in range(4):
                        iA = iAs[j]
                        iB = iBs[j]
                        cosF = etw_s[:, 8 + j:9 + j]
                        sinF = etw_s[:, 12 + j:13 + j]
                        pP = inv2("DiR", "DiIN", SF_R, SF_I, 4, j)
                        pQ = inv2("DiI", "DiR", SF_R, SF_I, 4, j)
                        tq = tpool.tile([128, C], f32, tag="tf")
                        nc.scalar.mul(tq, pQ, sinF)
                        n_lo = ipool.tile([128, C], mdt, tag="iv")
                        nc.vector.scalar_tensor_tensor(
                            out=n_lo, in0=pP, scalar=cosF,
                            in1=tq, op0=_ALU.mult, op1=_ALU.add)
                        tp2 = tpool.tile([128, C], f32, tag="tf")
                        nc.scalar.mul(tp2, pP, sinF)
                        n_hi = ipool.tile([128, C], mdt, tag="iv")
                        nc.vector.scalar_tensor_tensor(
                            out=n_hi, in0=pQ, scalar=cosF,
                            in1=tp2, op0=_ALU.mult, op1=_ALU.subtract)

                        pDre = inv2("DiR", "DiIN", SD_R, SD_I, 4, j)
                        iDre = ipool.tile([128, C], mdt, tag="iv")
                        nc.scalar.copy(out=iDre, in_=pDre)
                        pDim = inv2("DiI", "DiR", SD_R, SD_I, 4, j)
                        iDim = ipool.tile([128, C], mdt, tag="iv")
                        nc.scalar.copy(out=iDim, in_=pDim)

                        cosE = etw_s[:, j:j + 1]
                        sinE = etw_s[:, 4 + j:5 + j]
                        pP = inv2("DiR", "DiIN", SE_R, SE_I, 4, j)
                        pQ = inv2("DiI", "DiR", SE_R, SE_I, 4, j)
                        tq2 = tpool.tile([128, C], f32, tag="tf")
                        nc.scalar.mul(tq2, pQ, sinE)
                        iEre = ipool.tile([128, C], mdt, tag="iv")
                        nc.vector.scalar_tensor_tensor(
                            out=iEre, in0=pP, scalar=cosE,
                            in1=tq2, op0=_ALU.mult, op1=_ALU.add)
                        tp3 = tpool.tile([128, C], f32, tag="tf")
                        nc.scalar.mul(tp3, pP, sinE)
                        iEim = ipool.tile([128, C], mdt, tag="iv")
                        nc.vector.scalar_tensor_tensor(
                            out=iEim, in0=pQ, scalar=cosE,
                            in1=tp3, op0=_ALU.mult, op1=_ALU.subtract)

                        # combine into the 4 output quarters
                        sP = tpool.tile([128, C], mdt, tag="tb")
                        sM = tpool.tile([128, C], mdt, tag="tb")
                        nc.vector.tensor_add(sP, iA, iB)
                        nc.vector.tensor_sub(sM, iA, iB)
                        u = tpool.tile([128, C], mdt, tag="tb")
                        w2 = tpool.tile([128, C], mdt, tag="tb")
                        nc.vector.tensor_add(u, iDre, iEre)
                        nc.gpsimd.tensor_add(w2, iDim, iEim)
                        dR = tpool.tile([128, C], mdt, tag="tb")
                        dI = tpool.tile([128, C], mdt, tag="tb")
                        nc.vector.tensor_sub(dR, iDre, iEre)
                        nc.vector.tensor_sub(dI, iDim, iEim)
                        rotR = tpool.tile([128, C], mdt, tag="tb")
                        rotI = tpool.tile([128, C], mdt, tag="tb")
                        tsm = tpool.tile([128, C], mdt, tag="tb")
                        tdf = tpool.tile([128, C], mdt, tag="tb")
                        nc.vector.tensor_add(tsm, dR, dI)
                        nc.gpsimd.tensor_sub(tdf, dI, dR)
                        nc.scalar.mul(rotR, tsm, R2)
                        nc.scalar.mul(rotI, tdf, R2)
                        for q, (pm, nn1, ex, neg) in enumerate((
                                (sP, n_lo, u, False), (sM, n_hi, rotR, False),
                                (sP, n_lo, w2, True), (sM, n_hi, rotI, True))):
                            row0 = q * 512 + j * 128
                            g = gpool.tile([128, C], mdt, tag="g")
                            nc.sync.dma_start(
                                out=g, in_=p_scr[st, row0:row0 + 128, :])
                            x1 = tpool.tile([128, C], mdt, tag="tb")
                            if neg:
                                em().tensor_sub(x1, pm, nn1)
                            else:
                                em().tensor_add(x1, pm, nn1)
                            x2 = tpool.tile([128, C], mdt, tag="tb")
                            em().tensor_add(x2, x1, ex)
                            if st < ORDER - 1:
                                em().tensor_mul(v_buf[:, q * 4 + j, :], x2, g)
                            else:
                                og = tpool.tile([128, C], f32, tag="tf")
                                em().tensor_mul(og, x2, g)
                                nc.sync.dma_start(
                                    out=out_d[row0:row0 + 128, :], in_=og)


_PROGRAMS = {}


def build_program(mode=None, repeat=None):
    mode = MM_MODE if mode is None else mode
    repeat = REPEAT if repeat is None else repeat
    key = (mode, repeat)
    if key not in _PROGRAMS:
        nc = bacc.Bacc("TRN2", target_bir_lowering=False, debug=False,
                       enable_asserts=False, num_devices=N_CORES)
        _emit(nc, mode, repeat)
        nc.compile()
        _PROGRAMS[key] = nc
    return _PROGRAMS[key]


# ---------------- host-side tables ----------------

_TABLES = {}


def _lhsT_pack(Ms, npdt):
    """Pack matrices (in OFF order) into [128, N_MTILES*128] lhsT tiles.

    Tile (ot, it) column-block holds M[128*ot+q, 128*it+p] at [p, q]."""
    blocks = []
    for name in _MATS_SHAPES:
        M = Ms[name]
        o, i = M.shape
        assert (o, i) == _MATS_SHAPES[name], name
        tl = M.reshape(o // 128, 128, i // 128, 128).transpose(0, 2, 3, 1)
        blocks.append(tl.reshape(-1, 128, 128))
    big = np.concatenate(blocks, axis=0)          # [N_MTILES, 128(p), 128(q)]
    assert big.shape[0] == N_MTILES
    return np.ascontiguousarray(
        big.transpose(1, 0, 2).reshape(128, N_MTILES * 128).astype(npdt))


def host_tables(mode=None):
    mode = MM_MODE if mode is None else mode
    if mode in _TABLES:
        return _TABLES[mode]
    npdt = _np_mdt(mode)
    t5 = np.arange(512.0)
    tt5 = t5.reshape(1, -1)
    Ms = {}
    # A: packed cyclic-512
    kA = np.arange(256.0).reshape(-1, 1)
    angA = 2.0 * np.pi * kA * tt5 / 512.0
    Ms["AfC"] = np.cos(angA)
    Ms["AfS"] = -np.sin(angA)
    Ms["AfS"][0, :] = (-1.0) ** t5
    wA = np.full(256, 2.0 / 512.0)
    wA[0] = 1.0 / 512.0
    Ms["AiC"] = np.cos(angA).T * wA
    Ms["AiS"] = -np.sin(angA).T * (2.0 / 512.0)
    Ms["AiS"][:, 0] = ((-1.0) ** t5) / 512.0
    # B: nega-512
    kB = (np.arange(256.0) + 0.5).reshape(-1, 1)
    angB = 2.0 * np.pi * kB * tt5 / 512.0
    Ms["BfC"] = np.cos(angB)
    Ms["BfS"] = -np.sin(angB)
    Ms["BiC"] = np.cos(angB).T * (2.0 / 512.0)
    Ms["BiS"] = -np.sin(angB).T * (2.0 / 512.0)
    # D-family: mod(z^512 - e^{i pi/4}), alpha = 1/8 (shared by F, D, E)
    kD = (np.arange(512.0) + 0.125).reshape(-1, 1)
    angD = 2.0 * np.pi * kD * tt5 / 512.0
    Ms["DfR"] = np.cos(angD)
    Ms["DfI"] = np.sin(angD)
    Ms["DfIN"] = -np.sin(angD)
    Ms["DiR"] = np.cos(angD).T / 512.0
    Ms["DiI"] = -np.sin(angD).T / 512.0
    Ms["DiIN"] = np.sin(angD).T / 512.0
    mats = _lhsT_pack(Ms, npdt)

    p = np.arange(128.0)
    etw = np.zeros((128, 16), np.float32)
    for j in range(4):
        t = 128 * j + p
        etw[:, j] = np.cos(np.pi * t / 512.0)
        etw[:, 4 + j] = np.sin(np.pi * t / 512.0)
        etw[:, 8 + j] = np.cos(np.pi * t / 2048.0)
        etw[:, 12 + j] = np.sin(np.pi * t / 2048.0)

    _TABLES[mode] = {"mats": mats, "etw": etw}
    return _TABLES[mode]


def filter_spectra_core(filter_time, c0, npdt):
    """Branch spectra for channels [c0, c0+C): hspec [ORDER*32, 128, C] and
    hfix [1, ORDER*C] (A-branch Nyquist row)."""
    hs = np.zeros((ORDER * 32, 128, C), np.float64)
    hf = np.zeros((ORDER, C), np.float64)
    t5 = np.arange(512.0).reshape(1, -1)
    for st in range(ORDER):
        h = np.asarray(filter_time[st, 0], np.float64)[:, c0:c0 + C]
        h0, h1, h2, h3 = h[0:512], h[512:1024], h[1024:1536], h[1536:2048]
        hA = h0 + h1 + h2 + h3
        hB = h0 - h1 + h2 - h3
        s = np.exp(1j * np.pi / 4)
        hD = (h0 + 1j * h2) + s * (h1 + 1j * h3)
        hE = (h0 + 1j * h2) - s * (h1 + 1j * h3)

        FA = np.fft.fft(hA, n=512, axis=0)
        HA_r = FA[:256].real * 0.125
        HA_i = FA[:256].imag * 0.125
        hf[st] = FA[256].real * 0.125

        kB = (np.arange(256.0) + 0.5).reshape(-1, 1)
        WB = np.exp(-2j * np.pi * kB * t5 / 512.0)
        HB = (WB @ hB) * 0.125

        kF = (np.arange(512.0) + 0.25).reshape(-1, 1)
        WF = np.exp(2j * np.pi * kF * t5 / 512.0)
        HF = (WF @ ((h0 - h2) + 1j * (h1 - h3))) * 0.25

        kD_ = (np.arange(512.0) + 0.125).reshape(-1, 1)
        WD = np.exp(2j * np.pi * kD_ * t5 / 512.0)
        HD = (WD @ hD) * 0.25
        kE = (np.arange(512.0) + 0.625).reshape(-1, 1)
        WE = np.exp(2j * np.pi * kE * t5 / 512.0)
        HE = (WE @ hE) * 0.25

        def put(slot, arr):
            n = arr.shape[0] // 128
            hs[st * 32 + slot: st * 32 + slot + n] = arr.reshape(n, 128, C)

        put(SA_R, HA_r)
        put(SA_I, HA_i)
        put(SB_R, HB.real)
        put(SB_I, HB.imag)
        put(SF_R, HF.real)
        put(SF_I, HF.imag)
        put(SD_R, HD.real)
        put(SD_I, HD.imag)
        put(SE_R, HE.real)
        put(SE_I, HE.imag)
    return (hs.astype(npdt),
            np.ascontiguousarray(hf.reshape(1, ORDER * C).astype(np.float32)))


def make_in_maps(x, proj_w, proj_b, filter_time, mode=None):
    mode = MM_MODE if mode is None else mode
    npdt = _np_mdt(mode)
    tables = host_tables(mode)
    in_maps = []
    spectra_cache = {}
    for core in range(N_CORES):
        b, half = divmod(core, N_CORES // B)
        c0 = half * C
        if c0 not in spectra_cache:
            spectra_cache[c0] = filter_spectra_core(filter_time, c0, npdt)
        hs, hf = spectra_cache[c0]
        cols = np.concatenate(
            [np.arange(s * D + c0, s * D + c0 + C) for s in range(ORDER + 1)])
        in_maps.append({
            "xt": np.ascontiguousarray(np.asarray(x[b]).T.astype(npdt)),
            "w": np.ascontiguousarray(np.asarray(proj_w)[:, cols].astype(npdt)),
            "bias": np.ascontiguousarray(
                np.asarray(proj_b)[cols].astype(npdt).reshape(1, E)),
            "mats": tables["mats"],
            "etw": tables["etw"],
            "hspec": hs,
            "hfix": hf,
        })
    return in_maps


def gather_out(results):
    out = np.zeros((B, L, D), dtype=np.float32)
    for core in range(N_CORES):
        b, half = divmod(core, N_CORES // B)
        c0 = half * C
        out[b, :, c0:c0 + C] = results[core]["out"]
    return out


def kernel(x, proj_w, proj_b, filter_time):
    x = np.asarray(x)
    proj_w = np.asarray(proj_w)
    proj_b = np.asarray(proj_b)
    filter_time = np.asarray(filter_time)
    nc = build_program()
    in_maps = make_in_maps(x, proj_w, proj_b, filter_time)
    res = run_bass_kernel_spmd(nc, in_maps, list(range(N_CORES)))
    return gather_out(res.results)
